# revision 10
# baseline (speedup 1.0000x reference)
"""CrossAttentionBlock kernel for 8 Trainium2 NeuronCores — v2.

Sharding: 8 cores = 2 batches x 4 head-groups (4 heads each). Each core
computes the partial output of its 4 heads for its batch; host sums the
4 partials per batch and adds the bias.

v2 design (vs v1 baseline):
  - bf16 activations/weights (host converts); PSUM accumulation fp32.
  - Attention inner loop software-pipelined: scores(j) [PE], exp(j)
    [ACT] and AV(j-2) [PE] overlap via a double-buffered score PSUM
    ring and a 4-deep probability ring.
  - All projection matmuls (Q/K/V/out-proj) are emitted as fine-grained
    "filler" work interleaved into the attention loop so the PE never
    idles (the PE needs ~3us of continuous busy for full clock).
  - Softmax normalization: ones-column in the augmented V puts the
    row-sum in PSUM partition 64; DVE reciprocal stays partition-
    aligned, GpSimd partition_broadcast fans it across partitions, DVE
    multiplies, and a small SBUF->SBUF DMA places the half-tiles into
    the stacked normalized-output layout.

Per-core tensors (host prepares, bf16):
    xT  [1024, 2048]  x[b].T
    yT  [768, 2048]   y[b].T
    wq  [1024, 256]   Wq columns of this head group
    wk  [768, 256]    K-half of Wkv for this head group
    wv  [768, 256]    V-half of Wkv for this head group
    wp  [256, 1024]   Wproj rows of this head group
Output:
    outT [1024, 2048] fp32 partial (x @ .. @ Wproj).T for this head group
"""

import numpy as np
import ml_dtypes

import concourse.bass as bass
import concourse.tile as tile
from concourse import bacc, mybir
from concourse.bass_utils import run_bass_kernel_spmd

B, LQ, LKV = 2, 2048, 2048
C, CTX, H, DK = 1024, 768, 16, 64
SCALE = DK ** (-0.5)

F32 = mybir.dt.float32
F32R = mybir.dt.float32r
BF16 = mybir.dt.bfloat16

NCC = C // 128     # 8 contraction chunks for Q proj
NCTX = CTX // 128  # 6 contraction chunks for K/V proj
NIT = LQ // 512    # 4 query tiles
NJT = LKV // 128   # 16 key chunks
NCT = C // 128     # 8 output column tiles
HD = 256           # head-group width (4 heads x 64)

AV_LAG = 2         # AV(j - AV_LAG) emitted at loop step j


def build_kernel(debug=False):
    nc = bacc.Bacc("TRN2", target_bir_lowering=False, debug=False)

    xT = nc.dram_tensor("xT", [C, LQ], BF16, kind="ExternalInput").ap()
    yT = nc.dram_tensor("yT", [CTX, LKV], BF16, kind="ExternalInput").ap()
    wq = nc.dram_tensor("wq", [C, HD], BF16, kind="ExternalInput").ap()
    wk = nc.dram_tensor("wk", [CTX, HD], BF16, kind="ExternalInput").ap()
    wv = nc.dram_tensor("wv", [CTX, HD], BF16, kind="ExternalInput").ap()
    wp = nc.dram_tensor("wp", [HD, C], BF16, kind="ExternalInput").ap()
    outT = nc.dram_tensor("outT", [C, LQ], BF16, kind="ExternalOutput").ap()

    with tile.TileContext(nc) as tc:
        with (
            tc.tile_pool(name="sb", bufs=1) as sbp,
            tc.tile_pool(name="ptq", bufs=4) as ptp,
            tc.tile_pool(name="rs", bufs=6) as rsp,
            tc.tile_pool(name="bcp", bufs=6) as bcp,
            tc.tile_pool(name="osb", bufs=6) as osbp,
            tc.tile_pool(name="st", bufs=2, space="PSUM") as stp,
            tc.tile_pool(name="ot", bufs=2, space="PSUM") as otp,
            tc.tile_pool(name="pp", bufs=2, space="PSUM") as ppp,
        ):
            # ---- persistent SBUF
            x_sb = sbp.tile([128, NCC, LQ], BF16, tag="x")
            y_sb = sbp.tile([128, NCTX, LKV], BF16, tag="y")
            wq_sb = sbp.tile([128, NCC, HD], BF16, tag="wq")
            wk_sb = sbp.tile([128, NCTX, HD], BF16, tag="wk")
            wv_sb = sbp.tile([128, NCTX, HD], BF16, tag="wv")
            wp_sb = sbp.tile([128, 2, C], BF16, tag="wp")
            qt = sbp.tile([128, 2, LQ], BF16, tag="qt")       # Q^T pair-stacked
            kt = sbp.tile([128, 2, LKV], BF16, tag="kt")      # K^T pair-stacked
            vaug = sbp.tile([128, NJT, 4, 65], BF16, tag="vaug")  # [V_h | 1]
            otn = sbp.tile([128, 2, LQ], BF16, tag="otn")     # normalized O^T
            ones_sb = sbp.tile([65, 64], F32, tag="ones")

            nc.vector.memset(vaug[:], 1.0)   # col 64 survives as the ones col

            # ---- input DMAs (SP queue; order tuned for earliest compute)
            xr = xT.rearrange("(cc p) l -> p cc l", p=128)
            yr = yT.rearrange("(cc p) l -> p cc l", p=128)
            wkr = wk.rearrange("(c p) h -> p c h", p=128)
            nc.sync.dma_start(out=wk_sb[:, :, 0:128], in_=wkr[:, :, 0:128])
            nc.sync.dma_start(out=y_sb[:, :, 0:256], in_=yr[:, :, 0:256])
            nc.sync.dma_start(out=wv_sb, in_=wv.rearrange("(c p) h -> p c h", p=128))
            nc.sync.dma_start(out=y_sb[:, :, 256:512], in_=yr[:, :, 256:512])
            nc.sync.dma_start(out=wq_sb, in_=wq.rearrange("(c p) h -> p c h", p=128))
            nc.sync.dma_start(out=x_sb[:, :, 0:512], in_=xr[:, :, 0:512])
            nc.sync.dma_start(out=y_sb[:, :, 512:1024], in_=yr[:, :, 512:1024])
            nc.sync.dma_start(out=y_sb[:, :, 1024:1536], in_=yr[:, :, 1024:1536])
            nc.sync.dma_start(out=y_sb[:, :, 1536:2048], in_=yr[:, :, 1536:2048])
            for i in range(1, 4):
                nc.sync.dma_start(out=x_sb[:, :, i * 512:(i + 1) * 512],
                                  in_=xr[:, :, i * 512:(i + 1) * 512])
            nc.sync.dma_start(out=wk_sb[:, :, 128:256], in_=wkr[:, :, 128:256])
            nc.sync.dma_start(out=wp_sb, in_=wp.rearrange("(r p) o -> p r o", p=128))

            # ---- burst generators: yield True after each PE matmul,
            #      False after non-PE steps (copies).
            def alloc_ps(width=512):
                ps_t = ppp.tile([128, 512], F32, tag="pp", name="pps")
                return ps_t[:, 0:width]

            def proj_burst(dst, stat, mov, nred, width=512, psum="auto"):
                if psum == "pp":
                    ps_t = ppp.tile([128, 512], F32, tag="pp", name="ppf")
                    ps = ps_t[:, 0:width]
                else:
                    ps = alloc_ps(width)
                for cc in range(nred):
                    nc.tensor.matmul(ps, stat(cc), mov(cc),
                                     start=(cc == 0), stop=(cc == nred - 1))
                    yield True
                dst(ps)
                yield False

            def q_burst(pair, it, **kw):
                sl = slice(it * 512, (it + 1) * 512)
                return proj_burst(
                    lambda ps: nc.vector.tensor_copy(qt[:, pair, sl], ps),
                    lambda cc: wq_sb[:, cc, pair * 128:(pair + 1) * 128],
                    lambda cc: x_sb[:, cc, sl], NCC, **kw)

            def k_burst(pair, lc, lo=0, hi=512, **kw):
                sl = slice(lc * 512 + lo, lc * 512 + hi)
                return proj_burst(
                    lambda ps: nc.vector.tensor_copy(kt[:, pair, sl], ps),
                    lambda cc: wk_sb[:, cc, pair * 128:(pair + 1) * 128],
                    lambda cc: y_sb[:, cc, sl], NCTX, width=hi - lo, **kw)

            def v_burst(jt, **kw):
                return proj_burst(
                    lambda ps: nc.vector.tensor_copy(
                        vaug[:, jt, :, 0:64],
                        ps.rearrange("p (h d) -> p h d", d=64)),
                    lambda cc: y_sb[:, cc, jt * 128:(jt + 1) * 128],
                    lambda cc: wv_sb[:, cc, :], NCTX, width=256, **kw)

            def d_out(ps, ct, sl, eng=None):
                o_t = osbp.tile([128, 512], BF16, tag="osb", name="ot_o")
                nc.vector.tensor_copy(o_t[:], ps)
                nc.sync.dma_start(out=outT[ct * 128:(ct + 1) * 128, sl],
                                  in_=o_t[:])

            def d_burst(it, ct0=0, ct1=NCT, eng=None):
                sl = slice(it * 512, (it + 1) * 512)
                for ct in range(ct0, ct1):
                    ps_t = ppp.tile([128, 512], F32, tag="pp", name="ppd")
                    ps = ps_t[:]
                    for pair in range(2):
                        nc.tensor.matmul(
                            ps, wp_sb[:, pair, ct * 128:(ct + 1) * 128],
                            otn[:, pair, sl], start=(pair == 0), stop=(pair == 1))
                        yield True
                    d_out(ps, ct, sl, eng)
                    yield False

            class Fill:
                def __init__(self):
                    self.q = []   # list of (name, gen)

                def add(self, *gens):
                    self.q.extend(gens)

                def pop(self, n=1):
                    got = 0
                    while got < n and self.q:
                        try:
                            if next(self.q[0][1]):
                                got += 1
                        except StopIteration:
                            self.q.pop(0)

                def require(self, name):
                    """Fully drain queued gens up to and including `name`."""
                    while any(nm == name for nm, _ in self.q):
                        try:
                            next(self.q[0][1])
                        except StopIteration:
                            self.q.pop(0)

                def drain(self):
                    while self.q:
                        self.pop(1)

            fillers = Fill()

            def run_now(g):
                for _ in g:
                    pass

            # ---- upfront: only what tile (0,0) needs at its start
            run_now(k_burst(0, 0, 0, 256))
            run_now(v_burst(0))
            run_now(v_burst(1))
            run_now(k_burst(0, 0, 256, 512))
            run_now(v_burst(2))
            run_now(v_burst(3))
            run_now(q_burst(0, 0))
            run_now(k_burst(0, 1))

            # queue order mirrors the require() order inside the tiles
            fillers.add((("v", 4), v_burst(4)), (("v", 5), v_burst(5)),
                        (("k", 0, 2), k_burst(0, 2)),
                        (("v", 6), v_burst(6)), (("v", 7), v_burst(7)),
                        (("v", 8), v_burst(8)), (("v", 9), v_burst(9)),
                        (("k", 0, 3), k_burst(0, 3)),
                        *[(("v", j), v_burst(j)) for j in range(10, 16)],
                        (("q", 1, 0), q_burst(1, 0)),
                        (("k", 1, 0), k_burst(1, 0)),
                        (("k", 1, 1), k_burst(1, 1)),
                        (("k", 1, 2), k_burst(1, 2)),
                        (("k", 1, 3), k_burst(1, 3)))

            # ---- attention: one rolling pipeline across all 8 tiles.
            # The scores/exp stream never pauses at tile boundaries; AV lags
            # by AV_LAG steps and each tile's normalization rides behind its
            # last AV while the next tile's scores already flow.
            TILE = [(i % 2, i // 2) for i in range(2 * NIT)]  # (pair, it)
            NT = len(TILE)
            ots = {}
            rings = {}

            def norm_tile(ta):
                pair, it = TILE[ta]
                sl = slice(it * 512, (it + 1) * 512)
                ot_a, ot_b = ots.pop(ta)
                rs_a = rsp.tile([1, 512], F32, tag="rs", name="rsa")
                rs_b = rsp.tile([1, 512], F32, tag="rs", name="rsb")
                nc.vector.reciprocal(out=rs_a[:], in_=ot_a[64:65, :])
                nc.vector.reciprocal(out=rs_b[:], in_=ot_b[64:65, :])
                bc_a = bcp.tile([64, 512], F32, tag="bc", name="bca")
                bc_b = bcp.tile([64, 512], F32, tag="bc", name="bcb")
                nc.gpsimd.partition_broadcast(bc_a[:], rs_a[:])
                nc.gpsimd.partition_broadcast(bc_b[:], rs_b[:])
                nc.vector.tensor_mul(otn[0:64, pair, sl], ot_a[0:64, :],
                                     bc_a[:])
                nc.vector.tensor_mul(otn[64:128, pair, sl], ot_b[0:64, :],
                                     bc_b[:])

            for g in range(NT * NJT + AV_LAG):
                ts, js = divmod(g, NJT)
                if ts < NT and js == 0:
                    pair, it = TILE[ts]
                    if pair == 0:
                        if it < NIT - 1:
                            fillers.add(
                                (("q", 0, it + 1), q_burst(0, it + 1)),
                                (("q", 1, it + 1), q_burst(1, it + 1)))
                        if 1 <= it < NIT - 1:
                            fillers.add((("d", it - 1), d_burst(it - 1)))
                        if it == NIT - 1:
                            fillers.add((("d", it - 1, "a"),
                                         d_burst(it - 1, 0, NCT // 2)))
                    fillers.require(("q", pair, it))
                if js != 0:
                    fillers.pop(1)
                if ts < NT:
                    pair, it = TILE[ts]
                    sl = slice(it * 512, (it + 1) * 512)
                    if js % 4 == 0:
                        fillers.require(("k", pair, js // 4))
                    jsl = slice(js * 128, (js + 1) * 128)
                    st = stp.tile([128, 2, 512], F32, tag="st", name="sts")
                    nc.tensor.matmul(st[:, 0, :], kt[0:64, pair, jsl],
                                     qt[0:64, pair, sl], start=True, stop=True)
                    nc.tensor.matmul(st[:, 1, :], kt[64:128, pair, jsl],
                                     qt[64:128, pair, sl], start=True, stop=True)
                    pt = ptp.tile([128, 2, 512], BF16, tag="pt", name="pts")
                    nc.scalar.activation(pt[:], st[:],
                                         mybir.ActivationFunctionType.Exp,
                                         scale=SCALE)
                    rings.setdefault(ts, []).append(pt)
                ga = g - AV_LAG
                if ga >= 0:
                    ta, ja = divmod(ga, NJT)
                    pair, it = TILE[ta]
                    fillers.require(("v", ja))
                    if ja == 0:
                        ots[ta] = (
                            otp.tile([65, 512], F32, tag="ot", name="ota"),
                            otp.tile([65, 512], F32, tag="ot", name="otb"))
                    ot_a, ot_b = ots[ta]
                    prev = rings[ta][ja]
                    nc.tensor.matmul(ot_a[:], vaug[:, ja, 2 * pair, :],
                                     prev[:, 0, :], start=(ja == 0),
                                     stop=(ja == NJT - 1))
                    nc.tensor.matmul(ot_b[:], vaug[:, ja, 2 * pair + 1, :],
                                     prev[:, 1, :], start=(ja == 0),
                                     stop=(ja == NJT - 1))
                    if ja == NJT - 1:
                        norm_tile(ta)
                        del rings[ta]
                        fillers.pop(1)
                        if pair == 1 and it == NIT - 1:
                            fillers.add(
                                (("d", it - 1, "b"),
                                 d_burst(it - 1, NCT // 2, NCT, eng="alt")),
                                (("d", it), d_burst(it, eng="alt")))
            fillers.drain()

    nc.compile()
    return nc


_NC_CACHE = {}


def _get_nc():
    if "nc" not in _NC_CACHE:
        _NC_CACHE["nc"] = build_kernel()
    return _NC_CACHE["nc"]


def _bf16(a):
    return np.ascontiguousarray(a).astype(ml_dtypes.bfloat16)


def make_in_maps(x, y, Wq, Wkv, Wproj):
    """Host-side sharding: core = b * 4 + hg (hg = 4-head group)."""
    x = np.asarray(x, dtype=np.float32)
    y = np.asarray(y, dtype=np.float32)
    Wq = np.asarray(Wq, dtype=np.float32)
    Wkv = np.asarray(Wkv, dtype=np.float32).reshape(CTX, 2, H, DK)
    Wproj = np.asarray(Wproj, dtype=np.float32)

    in_maps = []
    for core in range(8):
        b, hg = core // 4, core % 4
        hs = slice(4 * hg, 4 * hg + 4)
        in_maps.append({
            "xT": _bf16(x[b].T),
            "yT": _bf16(y[b].T),
            "wq": _bf16(Wq[:, 4 * hg * DK:(4 * hg + 4) * DK]),
            "wk": _bf16(Wkv[:, 0, hs, :].reshape(CTX, 4 * DK)),
            "wv": _bf16(Wkv[:, 1, hs, :].reshape(CTX, 4 * DK)),
            "wp": _bf16(Wproj[4 * hg * DK:(4 * hg + 4) * DK, :]),
        })
    return in_maps


def kernel(x, y, Wq, Wkv, Wproj, bproj):
    nc = _get_nc()
    in_maps = make_in_maps(x, y, Wq, Wkv, Wproj)
    res = run_bass_kernel_spmd(nc, in_maps, core_ids=list(range(8)))
    bproj = np.asarray(bproj, dtype=np.float32)
    out = np.empty((B, LQ, C), dtype=np.float32)
    for b in range(B):
        acc = res.results[4 * b]["outT"].astype(np.float32)
        for hg in range(1, 4):
            acc += res.results[4 * b + hg]["outT"].astype(np.float32)
        out[b] = acc.T + bproj
    return out


# revision 11
# speedup vs baseline: 1.0006x; 1.0006x over previous
"""CrossAttentionBlock kernel for 8 Trainium2 NeuronCores — v2.

Sharding: 8 cores = 2 batches x 4 head-groups (4 heads each). Each core
computes the partial output of its 4 heads for its batch; host sums the
4 partials per batch and adds the bias.

v2 design (vs v1 baseline):
  - bf16 activations/weights (host converts); PSUM accumulation fp32.
  - Attention inner loop software-pipelined: scores(j) [PE], exp(j)
    [ACT] and AV(j-2) [PE] overlap via a double-buffered score PSUM
    ring and a 4-deep probability ring.
  - All projection matmuls (Q/K/V/out-proj) are emitted as fine-grained
    "filler" work interleaved into the attention loop so the PE never
    idles (the PE needs ~3us of continuous busy for full clock).
  - Softmax normalization: ones-column in the augmented V puts the
    row-sum in PSUM partition 64; DVE reciprocal stays partition-
    aligned, GpSimd partition_broadcast fans it across partitions, DVE
    multiplies, and a small SBUF->SBUF DMA places the half-tiles into
    the stacked normalized-output layout.

Per-core tensors (host prepares, bf16):
    xT  [1024, 2048]  x[b].T
    yT  [768, 2048]   y[b].T
    wq  [1024, 256]   Wq columns of this head group
    wk  [768, 256]    K-half of Wkv for this head group
    wv  [768, 256]    V-half of Wkv for this head group
    wp  [256, 1024]   Wproj rows of this head group
Output:
    outT [1024, 2048] fp32 partial (x @ .. @ Wproj).T for this head group
"""

import numpy as np
import ml_dtypes

import concourse.bass as bass
import concourse.tile as tile
from concourse import bacc, mybir
from concourse.bass_utils import run_bass_kernel_spmd

B, LQ, LKV = 2, 2048, 2048
C, CTX, H, DK = 1024, 768, 16, 64
SCALE = DK ** (-0.5)

F32 = mybir.dt.float32
F32R = mybir.dt.float32r
BF16 = mybir.dt.bfloat16

NCC = C // 128     # 8 contraction chunks for Q proj
NCTX = CTX // 128  # 6 contraction chunks for K/V proj
NIT = LQ // 512    # 4 query tiles
NJT = LKV // 128   # 16 key chunks
NCT = C // 128     # 8 output column tiles
HD = 256           # head-group width (4 heads x 64)

AV_LAG = 2         # AV(j - AV_LAG) emitted at loop step j


def build_kernel(debug=False):
    nc = bacc.Bacc("TRN2", target_bir_lowering=False, debug=False)

    xT = nc.dram_tensor("xT", [C, LQ], BF16, kind="ExternalInput").ap()
    yT = nc.dram_tensor("yT", [CTX, LKV], BF16, kind="ExternalInput").ap()
    wq = nc.dram_tensor("wq", [C, HD], BF16, kind="ExternalInput").ap()
    wk = nc.dram_tensor("wk", [CTX, HD], BF16, kind="ExternalInput").ap()
    wv = nc.dram_tensor("wv", [CTX, HD], BF16, kind="ExternalInput").ap()
    wp = nc.dram_tensor("wp", [HD, C], BF16, kind="ExternalInput").ap()
    outT = nc.dram_tensor("outT", [C, LQ], BF16, kind="ExternalOutput").ap()

    with tile.TileContext(nc) as tc:
        with (
            tc.tile_pool(name="sb", bufs=1) as sbp,
            tc.tile_pool(name="ptq", bufs=8) as ptp,
            tc.tile_pool(name="rs", bufs=8) as rsp,
            tc.tile_pool(name="bcp", bufs=8) as bcp,
            tc.tile_pool(name="osb", bufs=8) as osbp,
            tc.tile_pool(name="st", bufs=2, space="PSUM") as stp,
            tc.tile_pool(name="ot", bufs=2, space="PSUM") as otp,
            tc.tile_pool(name="pp", bufs=2, space="PSUM") as ppp,
        ):
            # ---- persistent SBUF
            x_sb = sbp.tile([128, NCC, LQ], BF16, tag="x")
            y_sb = sbp.tile([128, NCTX, LKV], BF16, tag="y")
            wq_sb = sbp.tile([128, NCC, HD], BF16, tag="wq")
            wk_sb = sbp.tile([128, NCTX, HD], BF16, tag="wk")
            wv_sb = sbp.tile([128, NCTX, HD], BF16, tag="wv")
            wp_sb = sbp.tile([128, 2, C], BF16, tag="wp")
            qt = sbp.tile([128, 2, LQ], BF16, tag="qt")       # Q^T pair-stacked
            kt = sbp.tile([128, 2, LKV], BF16, tag="kt")      # K^T pair-stacked
            vaug = sbp.tile([128, NJT, 4, 65], BF16, tag="vaug")  # [V_h | 1]
            otn = sbp.tile([128, 2, LQ], BF16, tag="otn")     # normalized O^T
            ones_sb = sbp.tile([65, 64], F32, tag="ones")

            nc.vector.memset(vaug[:], 1.0)   # col 64 survives as the ones col

            # ---- input DMAs (SP queue; order tuned for earliest compute)
            xr = xT.rearrange("(cc p) l -> p cc l", p=128)
            yr = yT.rearrange("(cc p) l -> p cc l", p=128)
            wkr = wk.rearrange("(c p) h -> p c h", p=128)
            nc.sync.dma_start(out=wk_sb[:, :, 0:128], in_=wkr[:, :, 0:128])
            nc.sync.dma_start(out=y_sb[:, :, 0:256], in_=yr[:, :, 0:256])
            nc.sync.dma_start(out=wv_sb, in_=wv.rearrange("(c p) h -> p c h", p=128))
            nc.sync.dma_start(out=y_sb[:, :, 256:512], in_=yr[:, :, 256:512])
            wqr = wq.rearrange("(c p) h -> p c h", p=128)
            nc.sync.dma_start(out=wq_sb[:, :, 0:128], in_=wqr[:, :, 0:128])
            nc.sync.dma_start(out=x_sb[:, :, 0:512], in_=xr[:, :, 0:512])
            nc.sync.dma_start(out=y_sb[:, :, 512:1024], in_=yr[:, :, 512:1024])
            nc.sync.dma_start(out=y_sb[:, :, 1024:1536], in_=yr[:, :, 1024:1536])
            nc.sync.dma_start(out=y_sb[:, :, 1536:2048], in_=yr[:, :, 1536:2048])
            for i in range(1, 4):
                nc.sync.dma_start(out=x_sb[:, :, i * 512:(i + 1) * 512],
                                  in_=xr[:, :, i * 512:(i + 1) * 512])
            nc.sync.dma_start(out=wk_sb[:, :, 128:256], in_=wkr[:, :, 128:256])
            nc.sync.dma_start(out=wq_sb[:, :, 128:256], in_=wqr[:, :, 128:256])
            nc.sync.dma_start(out=wp_sb, in_=wp.rearrange("(r p) o -> p r o", p=128))

            # ---- burst generators: yield True after each PE matmul,
            #      False after non-PE steps (copies).
            def alloc_ps(width=512):
                ps_t = ppp.tile([128, 512], F32, tag="pp", name="pps")
                return ps_t[:, 0:width]

            def proj_burst(dst, stat, mov, nred, width=512, psum="auto"):
                if psum == "pp":
                    ps_t = ppp.tile([128, 512], F32, tag="pp", name="ppf")
                    ps = ps_t[:, 0:width]
                else:
                    ps = alloc_ps(width)
                for cc in range(nred):
                    nc.tensor.matmul(ps, stat(cc), mov(cc),
                                     start=(cc == 0), stop=(cc == nred - 1))
                    yield True
                dst(ps)
                yield False

            def q_burst(pair, it, **kw):
                sl = slice(it * 512, (it + 1) * 512)
                return proj_burst(
                    lambda ps: nc.vector.tensor_copy(qt[:, pair, sl], ps),
                    lambda cc: wq_sb[:, cc, pair * 128:(pair + 1) * 128],
                    lambda cc: x_sb[:, cc, sl], NCC, **kw)

            def k_burst(pair, lc, lo=0, hi=512, **kw):
                sl = slice(lc * 512 + lo, lc * 512 + hi)
                return proj_burst(
                    lambda ps: nc.vector.tensor_copy(kt[:, pair, sl], ps),
                    lambda cc: wk_sb[:, cc, pair * 128:(pair + 1) * 128],
                    lambda cc: y_sb[:, cc, sl], NCTX, width=hi - lo, **kw)

            def v_burst(jt, **kw):
                return proj_burst(
                    lambda ps: nc.vector.tensor_copy(
                        vaug[:, jt, :, 0:64],
                        ps.rearrange("p (h d) -> p h d", d=64)),
                    lambda cc: y_sb[:, cc, jt * 128:(jt + 1) * 128],
                    lambda cc: wv_sb[:, cc, :], NCTX, width=256, **kw)

            def d_out(ps, ct, sl, eng=None):
                o_t = osbp.tile([128, 512], BF16, tag="osb", name="ot_o")
                nc.vector.tensor_copy(o_t[:], ps)
                nc.sync.dma_start(out=outT[ct * 128:(ct + 1) * 128, sl],
                                  in_=o_t[:])

            def d_burst(it, ct0=0, ct1=NCT, eng=None):
                sl = slice(it * 512, (it + 1) * 512)
                for ct in range(ct0, ct1):
                    ps_t = ppp.tile([128, 512], F32, tag="pp", name="ppd")
                    ps = ps_t[:]
                    for pair in range(2):
                        nc.tensor.matmul(
                            ps, wp_sb[:, pair, ct * 128:(ct + 1) * 128],
                            otn[:, pair, sl], start=(pair == 0), stop=(pair == 1))
                        yield True
                    d_out(ps, ct, sl, eng)
                    yield False

            class Fill:
                def __init__(self):
                    self.q = []   # list of (name, gen)

                def add(self, *gens):
                    self.q.extend(gens)

                def pop(self, n=1):
                    got = 0
                    while got < n and self.q:
                        try:
                            if next(self.q[0][1]):
                                got += 1
                        except StopIteration:
                            self.q.pop(0)

                def require(self, name):
                    """Fully drain queued gens up to and including `name`."""
                    while any(nm == name for nm, _ in self.q):
                        try:
                            next(self.q[0][1])
                        except StopIteration:
                            self.q.pop(0)

                def drain(self):
                    while self.q:
                        self.pop(1)

            fillers = Fill()

            def run_now(g):
                for _ in g:
                    pass

            # ---- upfront: only what tile (0,0) needs at its start
            run_now(k_burst(0, 0, 0, 256))
            run_now(v_burst(0))
            run_now(v_burst(1))
            run_now(k_burst(0, 0, 256, 512))
            run_now(v_burst(2))
            run_now(v_burst(3))
            run_now(q_burst(0, 0))
            run_now(k_burst(0, 1))

            # queue order mirrors the require() order inside the tiles
            fillers.add((("v", 4), v_burst(4)), (("v", 5), v_burst(5)),
                        (("k", 0, 2), k_burst(0, 2)),
                        (("v", 6), v_burst(6)), (("v", 7), v_burst(7)),
                        (("v", 8), v_burst(8)), (("v", 9), v_burst(9)),
                        (("k", 0, 3), k_burst(0, 3)),
                        *[(("v", j), v_burst(j)) for j in range(10, 16)],
                        (("q", 1, 0), q_burst(1, 0)),
                        (("k", 1, 0), k_burst(1, 0)),
                        (("k", 1, 1), k_burst(1, 1)),
                        (("k", 1, 2), k_burst(1, 2)),
                        (("k", 1, 3), k_burst(1, 3)))

            # ---- attention: one rolling pipeline across all 8 tiles.
            # The scores/exp stream never pauses at tile boundaries; AV lags
            # by AV_LAG steps and each tile's normalization rides behind its
            # last AV while the next tile's scores already flow.
            TILE = [(i % 2, i // 2) for i in range(2 * NIT)]  # (pair, it)
            NT = len(TILE)
            ots = {}
            rings = {}

            def norm_tile(ta):
                pair, it = TILE[ta]
                sl = slice(it * 512, (it + 1) * 512)
                ot_a, ot_b = ots.pop(ta)
                rs_a = rsp.tile([1, 512], F32, tag="rs", name="rsa")
                rs_b = rsp.tile([1, 512], F32, tag="rs", name="rsb")
                nc.vector.reciprocal(out=rs_a[:], in_=ot_a[64:65, :])
                nc.vector.reciprocal(out=rs_b[:], in_=ot_b[64:65, :])
                bc_a = bcp.tile([64, 512], F32, tag="bc", name="bca")
                bc_b = bcp.tile([64, 512], F32, tag="bc", name="bcb")
                nc.gpsimd.partition_broadcast(bc_a[:], rs_a[:])
                nc.gpsimd.partition_broadcast(bc_b[:], rs_b[:])
                nc.vector.tensor_mul(otn[0:64, pair, sl], ot_a[0:64, :],
                                     bc_a[:])
                nc.vector.tensor_mul(otn[64:128, pair, sl], ot_b[0:64, :],
                                     bc_b[:])

            for g in range(NT * NJT + AV_LAG):
                ts, js = divmod(g, NJT)
                if ts < NT and js == 0:
                    pair, it = TILE[ts]
                    if pair == 0:
                        if it < NIT - 1:
                            fillers.add(
                                (("q", 0, it + 1), q_burst(0, it + 1)),
                                (("q", 1, it + 1), q_burst(1, it + 1)))
                        if 1 <= it < NIT - 1:
                            fillers.add((("d", it - 1), d_burst(it - 1)))
                        if it == NIT - 1:
                            fillers.add((("d", it - 1, "a"),
                                         d_burst(it - 1, 0, NCT // 2)))
                    fillers.require(("q", pair, it))
                if js != 0:
                    fillers.pop(1)
                if ts < NT:
                    pair, it = TILE[ts]
                    sl = slice(it * 512, (it + 1) * 512)
                    if js % 4 == 0:
                        fillers.require(("k", pair, js // 4))
                    jsl = slice(js * 128, (js + 1) * 128)
                    st = stp.tile([128, 2, 512], F32, tag="st", name="sts")
                    nc.tensor.matmul(st[:, 0, :], kt[0:64, pair, jsl],
                                     qt[0:64, pair, sl], start=True, stop=True)
                    nc.tensor.matmul(st[:, 1, :], kt[64:128, pair, jsl],
                                     qt[64:128, pair, sl], start=True, stop=True)
                    pt = ptp.tile([128, 2, 512], BF16, tag="pt", name="pts")
                    nc.scalar.activation(pt[:], st[:],
                                         mybir.ActivationFunctionType.Exp,
                                         scale=SCALE)
                    rings.setdefault(ts, []).append(pt)
                ga = g - AV_LAG
                if ga >= 0:
                    ta, ja = divmod(ga, NJT)
                    pair, it = TILE[ta]
                    fillers.require(("v", ja))
                    if ja == 0:
                        ots[ta] = (
                            otp.tile([65, 512], F32, tag="ot", name="ota"),
                            otp.tile([65, 512], F32, tag="ot", name="otb"))
                    ot_a, ot_b = ots[ta]
                    prev = rings[ta][ja]
                    nc.tensor.matmul(ot_a[:], vaug[:, ja, 2 * pair, :],
                                     prev[:, 0, :], start=(ja == 0),
                                     stop=(ja == NJT - 1))
                    nc.tensor.matmul(ot_b[:], vaug[:, ja, 2 * pair + 1, :],
                                     prev[:, 1, :], start=(ja == 0),
                                     stop=(ja == NJT - 1))
                    if ja == NJT - 1:
                        norm_tile(ta)
                        del rings[ta]
                        fillers.pop(1)
                        if pair == 1 and it == NIT - 1:
                            fillers.add(
                                (("d", it - 1, "b"),
                                 d_burst(it - 1, NCT // 2, NCT, eng="alt")),
                                (("d", it), d_burst(it, eng="alt")))
            fillers.drain()

    nc.compile()
    return nc


_NC_CACHE = {}


def _get_nc():
    if "nc" not in _NC_CACHE:
        _NC_CACHE["nc"] = build_kernel()
    return _NC_CACHE["nc"]


def _bf16(a):
    return np.ascontiguousarray(a).astype(ml_dtypes.bfloat16)


def make_in_maps(x, y, Wq, Wkv, Wproj):
    """Host-side sharding: core = b * 4 + hg (hg = 4-head group)."""
    x = np.asarray(x, dtype=np.float32)
    y = np.asarray(y, dtype=np.float32)
    Wq = np.asarray(Wq, dtype=np.float32)
    Wkv = np.asarray(Wkv, dtype=np.float32).reshape(CTX, 2, H, DK)
    Wproj = np.asarray(Wproj, dtype=np.float32)

    in_maps = []
    for core in range(8):
        b, hg = core // 4, core % 4
        hs = slice(4 * hg, 4 * hg + 4)
        in_maps.append({
            "xT": _bf16(x[b].T),
            "yT": _bf16(y[b].T),
            "wq": _bf16(Wq[:, 4 * hg * DK:(4 * hg + 4) * DK]),
            "wk": _bf16(Wkv[:, 0, hs, :].reshape(CTX, 4 * DK)),
            "wv": _bf16(Wkv[:, 1, hs, :].reshape(CTX, 4 * DK)),
            "wp": _bf16(Wproj[4 * hg * DK:(4 * hg + 4) * DK, :]),
        })
    return in_maps


def kernel(x, y, Wq, Wkv, Wproj, bproj):
    nc = _get_nc()
    in_maps = make_in_maps(x, y, Wq, Wkv, Wproj)
    res = run_bass_kernel_spmd(nc, in_maps, core_ids=list(range(8)))
    bproj = np.asarray(bproj, dtype=np.float32)
    out = np.empty((B, LQ, C), dtype=np.float32)
    for b in range(B):
        acc = res.results[4 * b]["outT"].astype(np.float32)
        for hg in range(1, 4):
            acc += res.results[4 * b + hg]["outT"].astype(np.float32)
        out[b] = acc.T + bproj
    return out
